# revision 2
# baseline (speedup 1.0000x reference)
"""DeepSeekMoE forward on 8 Trainium2 NeuronCores.

Sharding: expert-parallel. Core c owns expert group c (8 of 64 experts) and a
1/8 column slice of the shared expert. The gate is replicated; its expert axis
is permuted per-core (own group first) so all cores run one SPMD program.
Each core produces a partial-sum [T, H]; the host reduces the 8 partials.

v2 structural changes vs v1:
- combine weights folded into the transposed one-hot matrices (met_w holds
  w[tok] instead of 1.0), removing the slot-weight extraction matmuls and the
  per-slot output scale.
- shared-expert down-projection folded into the combine PSUM chain (one extra
  matmul per (tch, hh) accumulation), removing the 4MB yshf staging tile.
- token gather shares lhsT across 4-expert blocks: 512-col matmul chains.
- routing runs first with shared-expert l1 and one-hot transposes interleaved
  into its PE gaps (PE p-state: gaps halve the clock).
- one-hot builds (is_equal / weight mul) run on the vector engine in bf16
  (slot indices are exact in bf16 below 256); Pool/GPSIMD turned out ~10x
  slower per element and is left idle.
- xthi (bf16 x^T) derived on device from xtf (fp32 x^T) on the Act engine.
"""
import sys

sys.path.insert(0, "/opt/trn_rl_repo")

import numpy as np
import ml_dtypes
import orjson

import concourse.bass as bass
import concourse.mybir as mybir
from concourse.tile import TileContext
from concourse.masks import make_identity
from concourse.bass_utils import run_bass_kernel_spmd

F32 = mybir.dt.float32
BF16 = mybir.dt.bfloat16
BF = ml_dtypes.bfloat16

P = 128          # partitions / token chunk / capacity
T = 1024         # tokens
H = 1024         # hidden
II = 512         # expert intermediate
E = 64           # routed experts
EL = 8           # local experts per core
NC = 8           # cores
C = 128          # per-expert token capacity
NCH = T // P     # token chunks
KH = H // P      # contraction chunks over H
EGRP = 8         # experts per combine pass (single pass, shared folded in)


def _split_waits_json(bir_bytes: bytes, max_waits: int = 1) -> bytes:
    """This walrus build accepts at most one sync wait per instruction; hoist
    extras into standalone EventSemaphore instructions on the same engine."""
    d = orjson.loads(bir_bytes)
    for fn in d.get("functions", []):
        for blk in fn.get("blocks", []):
            out = []
            for inst in blk.get("instructions", []):
                si = inst.get("sync_info") or {}
                waits = si.get("on_wait") or []
                if len(waits) > max_waits:
                    for j, w in enumerate(waits[:-max_waits]):
                        out.append({
                            "debug": inst.get("debug", 0),
                            "engine": inst["engine"],
                            "ins": [], "outs": [],
                            "name": f"{inst['name']}_hw{j}",
                            "opcode": "EventSemaphore",
                            "sync_info": {"on_update": [], "on_wait": [w]},
                        })
                    si["on_wait"] = waits[-max_waits:]
                    inst["sync_info"] = si
                out.append(inst)
            blk["instructions"] = out
    return orjson.dumps(d)


def _build_program(repeat=1, debug=False):
    nc = bass.Bass("TRN2")
    AF = mybir.ActivationFunctionType

    # ---- I/O ----
    xtf_in = nc.dram_tensor("xtf", [P, KH * T], F32, kind="ExternalInput")
    xloc_in = nc.dram_tensor("xloc", [P, NCH * H], BF16, kind="ExternalInput")
    gtf_in = nc.dram_tensor("gtf", [P, KH * E], F32, kind="ExternalInput")
    wa_in = nc.dram_tensor("wbloba", [EL, P, 8192], BF16, kind="ExternalInput")
    wd_in = nc.dram_tensor("wblobd", [EL, P, 4096], BF16, kind="ExternalInput")
    shw_in = nc.dram_tensor("shw", [P, 3072], BF16, kind="ExternalInput")
    ltri_in = nc.dram_tensor("ltri", [P, P], F32, kind="ExternalInput")
    r127_in = nc.dram_tensor("r127", [P, P], F32, kind="ExternalInput")
    iotac_in = nc.dram_tensor("iotac", [P, C], BF16, kind="ExternalInput")
    bias_in = nc.dram_tensor("biasbc", [P, E], F32, kind="ExternalInput")
    out_d = nc.dram_tensor("out", [T, H], F32, kind="ExternalOutput")

    with TileContext(nc) as tc:
        with tc.tile_pool(name="cst", bufs=1) as cst, \
             tc.tile_pool(name="big", bufs=1) as big, \
             tc.tile_pool(name="wts", bufs=2) as wts, \
             tc.tile_pool(name="rt", bufs=2) as rt, \
             tc.tile_pool(name="ex", bufs=2) as ex, \
             tc.tile_pool(name="cmb", bufs=EGRP) as cmb, \
             tc.tile_pool(name="ppG", bufs=2, space="PSUM") as ppG, \
             tc.tile_pool(name="ppL", bufs=1, space="PSUM") as ppL, \
             tc.tile_pool(name="ppT", bufs=2, space="PSUM") as ppT, \
             tc.tile_pool(name="ppB", bufs=2, space="PSUM") as ppB:

            # ---- resident loads (routing-critical first) ----
            gtf = cst.tile([P, KH * E], F32)
            nc.sync.dma_start(gtf[:], gtf_in[:])
            xtf = big.tile([P, KH * T], F32)
            for kk in range(KH):
                nc.sync.dma_start(xtf[:, kk * T:(kk + 1) * T],
                                  xtf_in[:, kk * T:(kk + 1) * T])
            ltri = cst.tile([P, P], F32)
            nc.sync.dma_start(ltri[:], ltri_in[:])
            r127 = cst.tile([P, P], F32)
            nc.sync.dma_start(r127[:], r127_in[:])
            iotac = cst.tile([P, C], BF16)
            nc.sync.dma_start(iotac[:], iotac_in[:])
            biasbc = cst.tile([P, E], F32)
            nc.sync.dma_start(biasbc[:], bias_in[:])
            shw = cst.tile([P, 3072], BF16)
            nc.sync.dma_start(shw[:], shw_in[:])
            xloc = big.tile([P, NCH * H], BF16)
            nc.sync.dma_start(xloc[:], xloc_in[:])
            ident = cst.tile([P, P], BF16)
            make_identity(nc, ident[:])

            consts = (xtf, xloc, gtf, shw, ltri, r127, iotac, biasbc, ident)
            pools = (wts, rt, ex, cmb, ppG, ppL, ppT, ppB)
            for rep in range(repeat):
                _phase_body(nc, AF, rep, consts, (wa_in, wd_in, out_d), big,
                            pools, debug=debug)

    orig = nc.to_json_bytes
    nc.to_json_bytes = lambda: _split_waits_json(orig())
    return nc


def _phase_body(nc, AF, rep, consts, drams, big, pools, debug=False):
    (xtf, xloc, gtf, shw, ltri, r127, iotac, biasbc, ident) = consts
    (wa_in, wd_in, out_d) = drams
    (wts, rt, ex, cmb, ppG, ppL, ppT, ppB) = pools
    AL = mybir.AluOpType
    AX = mybir.AxisListType

    # expert weight streams, ring-prefetched
    wa = [wts.tile([P, 8192], BF16, tag="wexpa", name=f"wa{rep}_{i}")
          for i in range(EL)]
    wd = [wts.tile([P, 4096], BF16, tag="wexpd", name=f"wd{rep}_{i}")
          for i in range(EL)]
    for e in range(EL):
        for p in range(4):
            nc.sync.dma_start(wa[e][:, p * 2048:(p + 1) * 2048],
                                wa_in[e, :, p * 2048:(p + 1) * 2048])
        for p in range(2):
            nc.sync.dma_start(wd[e][:, p * 2048:(p + 1) * 2048],
                                wd_in[e, :, p * 2048:(p + 1) * 2048])

    # bf16 x^T derived from fp32 x^T (Act), interleaved with routing sigmoids
    xthi = big.tile([P, KH * T], BF16, tag="xthi", name=f"xthi{rep}")

    # persistent per-body tiles
    me_all = big.tile([P, NCH * EL * C], BF16, tag="me_all", name=f"me{rep}")
    met_w = big.tile([P, NCH * EL * C], BF16, tag="met_w", name=f"met{rep}")
    h2sh = big.tile([P, T], BF16, tag="h2sh", name=f"h2sh{rep}")
    slotb = big.tile([P, NCH * EL], F32, tag="slotb", name=f"slotb{rep}")
    wlob = big.tile([P, NCH * EL], F32, tag="wlob", name=f"wlob{rep}")
    runoff = big.tile([P, E], F32, tag="runoff", name=f"runoff{rep}")
    nc.vector.memset(runoff[:], 0.0)

    # ---- phase R: routing, with shared-expert l1 and one-hot transposes
    # interleaved into the PE gaps ----
    def shared_l1(th):
        pg = ppL.tile([P, 512], F32, tag="l1g")
        pu = ppL.tile([P, 512], F32, tag="l1u")
        for kk in range(KH):
            xs = xthi[:, kk * T + th * 512: kk * T + (th + 1) * 512]
            nc.tensor.matmul(pg[:], lhsT=shw[:, kk * P:(kk + 1) * P],
                             rhs=xs, start=(kk == 0), stop=(kk == KH - 1))
        for kk in range(KH):
            xs = xthi[:, kk * T + th * 512: kk * T + (th + 1) * 512]
            nc.tensor.matmul(
                pu[:], lhsT=shw[:, 1024 + kk * P: 1024 + (kk + 1) * P],
                rhs=xs, start=(kk == 0), stop=(kk == KH - 1))
        sa = ex.tile([P, 512], F32, tag="sact", name=f"ssa{rep}_{th}")
        nc.scalar.activation(sa[:], pg[:], AF.Silu)
        nc.vector.tensor_mul(h2sh[:, th * 512:(th + 1) * 512], sa[:], pu[:])

    def transposes(ch):
        # met_w[ch] = transpose(me_all[ch] * w) in 2 half-tiles
        mw = rt.tile([P, EL * C], BF16, tag="mw", bufs=1, name=f"mw{rep}_{ch}")
        for e in range(EL):
            nc.vector.tensor_scalar_mul(
                mw[:, e * C:(e + 1) * C],
                me_all[:, ch * EL * C + e * C: ch * EL * C + (e + 1) * C],
                wlob[:, ch * EL + e: ch * EL + e + 1])
        for half in range(2):
            tp = ppT.tile([P, 512], BF16, tag="tr")
            for q in range(4):
                nc.tensor.transpose(
                    tp[:, q * P:(q + 1) * P],
                    mw[:, half * 512 + q * P: half * 512 + (q + 1) * P],
                    ident[:])
            nc.scalar.activation(
                met_w[:, ch * EL * C + half * 512: ch * EL * C + half * 512 + 512],
                tp[:], AF.Copy)

    for ch in range(NCH):
        # gate logits (fp32: selection gaps are ~1e-5, bf16 flips picks)
        lg = ppG.tile([P, E], F32, tag="xg")
        for kk in range(KH):
            nc.tensor.matmul(
                lg[:], lhsT=xtf[:, kk * T + ch * P: kk * T + ch * P + P],
                rhs=gtf[:, kk * E:(kk + 1) * E],
                start=(kk == 0), stop=(kk == KH - 1))
        # scores = sigmoid(logits) + bias
        sig = rt.tile([P, E], F32, tag="sig")
        nc.scalar.activation(sig[:], lg[:], AF.Sigmoid)
        # xthi derive after this chunk's sigmoid on the Act queue; all eight
        # slices are written before shared_l1(0) at ch==4 reads them
        if ch < 4:
            for kk in (2 * ch, 2 * ch + 1):
                nc.scalar.activation(xthi[:, kk * T:(kk + 1) * T],
                                     xtf[:, kk * T:(kk + 1) * T], AF.Copy)
        nc.vector.tensor_add(sig[:], sig[:], biasbc[:])
        # group top-4 mask
        gmax = rt.tile([P, 8], F32, tag="gmax")
        nc.vector.tensor_reduce(
            out=gmax[:], in_=sig[:].rearrange("p (g e) -> p g e", e=8),
            op=AL.max, axis=AX.X)
        t8g = rt.tile([P, 8], F32, tag="t8g")
        nc.vector.max(out=t8g[:], in_=gmax[:])
        gmask = rt.tile([P, 8], F32, tag="gmask")
        nc.vector.tensor_scalar(gmask[:], gmax[:], t8g[:, 3:4], None,
                                op0=AL.is_ge)
        gmx = rt.tile([P, E], F32, tag="gmx")
        nc.vector.tensor_copy(gmx[:], gmask[:].unsqueeze(2)
                              .to_broadcast([P, 8, 8]))
        # masked scores, top-6 mask
        msc = rt.tile([P, E], F32, tag="msc")
        nc.vector.tensor_mul(msc[:], sig[:], gmx[:])
        t8e = rt.tile([P, 8], F32, tag="t8e")
        nc.vector.max(out=t8e[:], in_=msc[:])
        m6 = rt.tile([P, E], F32, tag="m6")
        nc.vector.tensor_scalar(m6[:], msc[:], t8e[:, 5:6], None,
                                op0=AL.is_ge)
        # normalized combine weights for the 8 local experts (bf16)
        cu = rt.tile([P, E], F32, tag="cu")
        nc.vector.tensor_mul(cu[:], msc[:], m6[:])
        den = rt.tile([P, 1], F32, tag="den")
        nc.vector.tensor_reduce(out=den[:], in_=cu[:], op=AL.add, axis=AX.X)
        nc.vector.tensor_scalar_add(den[:], den[:], 1e-8)
        rden = rt.tile([P, 1], F32, tag="rden")
        nc.vector.reciprocal(rden[:], den[:])
        wloc = rt.tile([P, EL], F32, tag="wloc")
        nc.vector.tensor_scalar_mul(wloc[:], cu[:, 0:EL], rden[:, 0:1])
        nc.vector.tensor_copy(wlob[:, ch * EL:(ch + 1) * EL], wloc[:])
        # capacity slots: masked_slot = (pref + runoff) * m6 - 1 (local 8 only)
        pf = ppG.tile([P, E], F32, tag="xg")
        nc.tensor.matmul(pf[:], lhsT=ltri[:], rhs=m6[:], start=True, stop=True)
        s0 = rt.tile([P, E], F32, tag="s0")
        nc.vector.tensor_add(s0[:], pf[:], runoff[:])
        s1 = rt.tile([P, EL], F32, tag="s1")
        nc.vector.tensor_mul(s1[:], s0[:, 0:EL], m6[:, 0:EL])
        nc.vector.tensor_scalar_sub(slotb[:, ch * EL:(ch + 1) * EL], s1[:], 1.0)
        # runoff = broadcast(row 127 of (pref + runoff))
        rb = ppG.tile([P, E], F32, tag="xg")
        nc.tensor.matmul(rb[:], lhsT=r127[:], rhs=s0[:], start=True, stop=True)
        nc.vector.tensor_copy(runoff[:], rb[:])
        # one-hot slot matrices for the 8 local experts (Pool engine, bf16)
        for e in range(EL):
            nc.vector.tensor_scalar(
                me_all[:, ch * EL * C + e * C: ch * EL * C + (e + 1) * C],
                iotac[:], slotb[:, ch * EL + e: ch * EL + e + 1],
                None, op0=AL.is_equal)

        if ch >= 1:
            transposes(ch - 1)
        if ch == 4:
            shared_l1(0)
        if ch == 6:
            shared_l1(1)
    transposes(NCH - 1)

    # ---- gather: xt[blk][:, hk*512 + j*128] = token-gathered x^T for the
    # 4 experts j of block blk (lhsT shared across the block) ----
    xt = []
    for blk in range(2):
        xtb = ex.tile([P, KH * 512], BF16, tag="xt", name=f"xt{rep}_{blk}")
        for hk in range(KH):
            gp = ppG.tile([P, 512], F32, tag="xg")
            for tch in range(NCH):
                nc.tensor.matmul(
                    gp[:], lhsT=xloc[:, tch * H + hk * P: tch * H + (hk + 1) * P],
                    rhs=me_all[:, tch * EL * C + blk * 512:
                               tch * EL * C + blk * 512 + 512],
                    start=(tch == 0), stop=(tch == NCH - 1))
            nc.scalar.activation(xtb[:, hk * 512:(hk + 1) * 512], gp[:],
                                 AF.Copy)
        xt.append(xtb)

    # ---- experts: l1 + swiglu + l3; one combine pass (shared + 8 experts
    # accumulated in PSUM per (tch, hh)), then store ----
    yscs = {}

    def combine():
        for tch in range(NCH):
            for hh in range(2):
                cp = ppB.tile([P, 512], F32, tag="l3")
                nc.tensor.matmul(
                    cp[:], lhsT=h2sh[:, tch * P:(tch + 1) * P],
                    rhs=shw[:, 2048 + hh * 512: 2048 + (hh + 1) * 512],
                    start=True, stop=False)
                for ge in range(EL):
                    nc.tensor.matmul(
                        cp[:],
                        lhsT=met_w[:, tch * EL * C + ge * C:
                                   tch * EL * C + (ge + 1) * C],
                        rhs=yscs[ge][:, hh * 512:(hh + 1) * 512],
                        start=False, stop=(ge == EL - 1))
                ob = rt.tile([P, 512], F32, tag="osb",
                             name=f"osb{rep}_{tch}_{hh}")
                nc.scalar.activation(ob[:], cp[:], AF.Copy)
                nc.sync.dma_start(
                    out_d[tch * P:(tch + 1) * P, hh * 512:(hh + 1) * 512],
                    ob[:])

    for e in range(EL):
        blk, j = e // 4, e % 4
        pg = ppL.tile([C, II], F32, tag="l1g")
        pu = ppL.tile([C, II], F32, tag="l1u")
        for kk in range(KH):
            nc.tensor.matmul(pg[:],
                             lhsT=xt[blk][:, kk * 512 + j * P: kk * 512 + (j + 1) * P],
                             rhs=wa[e][:, kk * II:(kk + 1) * II],
                             start=(kk == 0), stop=(kk == KH - 1))
        for kk in range(KH):
            nc.tensor.matmul(
                pu[:],
                lhsT=xt[blk][:, kk * 512 + j * P: kk * 512 + (j + 1) * P],
                rhs=wa[e][:, 4096 + kk * II: 4096 + (kk + 1) * II],
                start=(kk == 0), stop=(kk == KH - 1))
        sa = ex.tile([C, II], F32, tag="sact", name=f"sa{rep}_{e}")
        nc.scalar.activation(sa[:], pg[:], AF.Silu)
        h2 = ex.tile([C, II], BF16, tag="h2", name=f"h2{rep}_{e}")
        nc.vector.tensor_mul(h2[:], sa[:], pu[:])
        # transpose h2 -> [II, C]
        tp = ppT.tile([P, 4 * C], BF16, tag="tr")
        for kk in range(4):
            nc.tensor.transpose(tp[:, kk * P:(kk + 1) * P],
                                h2[:, kk * P:(kk + 1) * P], ident[:])
        h2t = ex.tile([P, 4 * C], BF16, tag="h2t", name=f"h2t{rep}_{e}")
        nc.scalar.activation(h2t[:], tp[:], AF.Copy)
        # l3: y = h2 @ Wd^T (unscaled; combine weights live in met_w)
        ysc = cmb.tile([C, H], BF16, tag="ysc", name=f"ysc{rep}_{e}")
        for hh in range(2):
            yp = ppB.tile([C, 512], F32, tag="l3")
            for kk in range(4):
                nc.tensor.matmul(
                    yp[:], lhsT=h2t[:, kk * P:(kk + 1) * P],
                    rhs=wd[e][:, kk * H + hh * 512: kk * H + (hh + 1) * 512],
                    start=(kk == 0), stop=(kk == 3))
            nc.scalar.activation(ysc[:, hh * 512:(hh + 1) * 512], yp[:],
                                 AF.Copy)
        yscs[e] = ysc
    combine()

    if debug:
        for nm, tl in [("d_me", me_all), ("d_met", met_w), ("d_slotb", slotb),
                       ("d_wlob", wlob), ("d_h2sh", h2sh), ("d_xthi", xthi),
                       ("d_xt0", xt[0]), ("d_xt1", xt[1]),
                       ("d_ysc0", yscs[0]), ("d_ysc7", yscs[7])]:
            dt = tl.dtype if hasattr(tl, "dtype") else BF16
            od = nc.dram_tensor(nm, list(tl.shape), dt, kind="ExternalOutput")
            nc.sync.dma_start(od[:, :], tl[:])


_PROG = None


def _pack(a):
    """[KH*P, F] -> [P, KH*F] with chunk kk at columns kk*F:(kk+1)*F."""
    kh = a.shape[0] // P
    return np.ascontiguousarray(
        a.reshape(kh, P, -1).transpose(1, 0, 2).reshape(P, -1))


def _prep_core_inputs(c, x, gate_w, gate_bias, eg_w, eu_w, ed_w, sg_w, su_w, sd_w):
    perm = [c] + [g for g in range(NC) if g != c]
    eperm = np.concatenate([np.arange(g * 8, g * 8 + 8) for g in perm])

    xT = np.ascontiguousarray(x.T)                       # [H, T]
    gT = np.ascontiguousarray(gate_w[eperm].T)           # [H, E]

    wbloba = np.empty((EL, P, 8192), BF)
    wblobd = np.empty((EL, P, 4096), BF)
    for e in range(EL):
        ge = c * 8 + e
        wbloba[e, :, 0:4096] = _pack(eg_w[ge].T.astype(BF))
        wbloba[e, :, 4096:8192] = _pack(eu_w[ge].T.astype(BF))
        wblobd[e] = _pack(ed_w[ge].T.astype(BF))

    sl = slice(c * P, (c + 1) * P)
    shw = np.empty((P, 3072), BF)
    shw[:, 0:1024] = _pack(sg_w[sl].T.astype(BF))
    shw[:, 1024:2048] = _pack(su_w[sl].T.astype(BF))
    shw[:, 2048:3072] = np.ascontiguousarray(sd_w[:, sl].T).astype(BF)

    return {
        "xtf": _pack(xT),
        "xloc": _pack(x.astype(BF)),
        "gtf": _pack(gT),
        "wbloba": wbloba, "wblobd": wblobd, "shw": shw,
        "ltri": np.triu(np.ones((P, P), np.float32)),
        "r127": np.concatenate([np.zeros((127, P), np.float32),
                                np.ones((1, P), np.float32)]),
        "iotac": np.broadcast_to(np.arange(C, dtype=np.float32),
                                 (P, C)).astype(BF),
        "biasbc": np.broadcast_to(
            gate_bias[eperm].astype(np.float32), (P, E)).copy(),
    }


def kernel(hidden_states, gate_w, gate_bias, eg_w, eu_w, ed_w, sg_w, su_w, sd_w):
    global _PROG
    if _PROG is None:
        _PROG = _build_program()
    nc = _PROG

    x = np.asarray(hidden_states, np.float32).reshape(T, H)
    args = [np.asarray(a, np.float32) for a in
            (gate_w, gate_bias, eg_w, eu_w, ed_w, sg_w, su_w, sd_w)]
    in_maps = [_prep_core_inputs(c, x, *args) for c in range(NC)]
    res = run_bass_kernel_spmd(nc, in_maps, list(range(NC)))
    out = np.zeros((T, H), np.float32)
    for c in range(NC):
        out += res.results[c]["out"]
    return out.reshape(1, T, H)
